# revision 5
# baseline (speedup 1.0000x reference)
"""Trainium2 Bass kernel for nn_Controller (batch-1 two-layer LSTM-cell chain
+ choice head), distributed over 8 NeuronCores.

Math notes: both LSTMCells run with zero initial state, so h @ W_hh.T == 0 and
the f-gate multiplies c=0.  Only the i/g/o thirds of each W_ih matter:
    gates = x @ W_ih.T + (b_ih + b_hh)
    h     = sigmoid(o) * tanh(sigmoid(i) * tanh(g))

Structure (one collective total):
  - Weights are stored fp8-e4m3 pre-scaled by S=128 on the host (keeps the
    0.02-sigma values out of the subnormal range); the matching 1/S descale is
    folded into the activation-function `scale` operand for free.
  - Layer 0 row-sharded: core k computes the 768 i/o/g gate rows of its h0
    chunk as a flipped GEMV (x column stationary -> 1-column weight loads)
    using 4x PE column-group tiling: groups at array cols {0,32,64,96} stream
    four weight tiles concurrently, landing gates on psum partitions
    {0,32,64,96}.  Bias rides the accumulation group via K=1 matmuls vs ones.
  - h0 [4x64] is transposed to contraction layout [128,2] with four K=1
    matmuls against a ones column.
  - Layer 1 contraction-sharded: core k multiplies its own h0 chunk into
    W_ih_1[:, chunk] producing partial pre-activations for ALL 6144 gates
    (again 4x column-tiled).  No inter-layer collective.
  - One bf16 AllGather of the partials; each core tree-reduces the 8 copies
    on DVE (fp32), adds bias, applies the LSTM activations across 128
    partitions, and computes the 19-logit head locally.
  - ~10 warm-up matmuls on scratch data run during the initial weight-DMA
    window so the PE HAM clock-gate is at 2.4 GHz when real work arrives.
"""

import os
import sys

import numpy as np
import ml_dtypes

for _p in ("/opt/trn_rl_repo", os.path.expanduser("~/.axon_site/_ro/trn_rl_repo")):
    if os.path.isdir(_p) and _p not in sys.path:
        sys.path.insert(0, _p)

import concourse.bass as bass
import concourse.bacc as bacc
import concourse.mybir as mybir
import concourse.tile as tile
from concourse.bass_utils import run_bass_kernel_spmd

H = 2048
NCORES = 8
C = H // NCORES          # 256: per-core h chunk
NK = H // 128            # 16 k-tiles
CH = 19                  # choice logits
G1 = 3 * H               # 6144 layer-1 gates (i,o,g)
S = 128.0                # fp8 weight pre-scale
DT = mybir.dt.float32
DTW = mybir.dt.bfloat16
DT8 = mybir.dt.float8e4
BF = ml_dtypes.bfloat16
F8 = ml_dtypes.float8_e4m3

W0_COLS = NK * 768               # fp8 cols, 4 chunks of 3072
W1_COLS = 2 * G1                 # c-major: col = c*6144 + g*1536 + m*512 + j
CHUNK = 3072
# gate order everywhere is (i, o, g) so the two sigmoids fuse into one op
AROW = np.array([0, 3 * H, 2 * H])


# --------------------------------------------------------------------------
# host-side layout prep
# --------------------------------------------------------------------------

def _l0_rows(k):
    """W_ih_0 rows for core k in psum-column order: 4 col-groups x
    [i(64)|o(64)|g(64)], group g covering local h0 elems 64g..64g+63."""
    out = []
    for g in range(4):
        e = k * C + 64 * g + np.arange(64)
        out.append(np.concatenate([e, 3 * H + e, 2 * H + e]))
    return np.concatenate(out)


def _l1_rows():
    """W_ih_1 rows in AllGather-buffer order f = g*1536 + m*512 + j, where
    post-readback G[p, a*16+t] = gate a of h1 elem t*128+p, a in (i,o,g)."""
    f = np.arange(G1)
    p, rem = f // 48, f % 48
    a, t = rem // 16, rem % 16
    return AROW[a] + t * 128 + p


def _host_prep(inputs):
    idx = int(np.asarray(inputs["input_idx"]).reshape(-1)[0])
    emb = np.asarray(inputs["embedding"], np.float32)
    x0 = emb[idx]
    x0T = np.ascontiguousarray(x0.reshape(NK, 128).T.astype(BF))  # [128,16]

    W0 = np.asarray(inputs["w_ih_0"], np.float32)
    W1 = np.asarray(inputs["w_ih_1"], np.float32)
    B0 = np.asarray(inputs["b_ih_0"], np.float32) + np.asarray(inputs["b_hh_0"], np.float32)
    B1 = np.asarray(inputs["b_ih_1"], np.float32) + np.asarray(inputs["b_hh_1"], np.float32)
    WC = np.asarray(inputs["w_choice"], np.float32)
    BC = np.asarray(inputs["b_choice"], np.float32)

    frows = _l1_rows()
    W1g = (W1[frows] * S).astype(F8)                     # [6144, 2048] fp8
    biasg = np.ascontiguousarray((B1[frows] * S).reshape(128, 48))

    wch = np.ascontiguousarray(
        np.transpose(WC.reshape(CH, NK, 128), (2, 1, 0)).reshape(128, NK * CH)
        .astype(BF))

    maps = []
    for k in range(NCORES):
        r0 = _l0_rows(k)
        blk = (W0[r0] * S).T.reshape(NK, 128, 768)       # [t, p, n] f32
        w0h = np.ascontiguousarray(
            np.transpose(blk, (1, 0, 2)).reshape(128, W0_COLS).astype(F8))

        b0h = np.ascontiguousarray(
            np.concatenate([B0[r0] * S, BC]).reshape(1, 787))

        # w1h[q, c*6144 + f] = S*W1[frows[f], k*256 + c*128 + q]
        sel = W1g[:, k * C:(k + 1) * C]                  # [6144, 256] fp8
        arr = sel.reshape(G1, 2, 128)                    # [f, c, q]
        w1h = np.ascontiguousarray(
            np.transpose(arr, (2, 1, 0)).reshape(128, W1_COLS))

        maps.append(dict(w0=w0h, w1=w1h, x0=x0T, p0c=b0h, biasg=biasg, wc=wch))
    return maps


# --------------------------------------------------------------------------
# device program (identical on all 8 cores; per-core data differs)
# --------------------------------------------------------------------------

def _build_nc():
    nc = bacc.Bacc("TRN2", target_bir_lowering=False, debug=False,
                   num_devices=NCORES)

    w0d = nc.dram_tensor("w0", [128, W0_COLS], DT8, kind="ExternalInput")
    w1d = nc.dram_tensor("w1", [128, W1_COLS], DT8, kind="ExternalInput")
    x0d = nc.dram_tensor("x0", [128, NK], DTW, kind="ExternalInput")
    p0cd = nc.dram_tensor("p0c", [1, 787], DT, kind="ExternalInput")
    biasgd = nc.dram_tensor("biasg", [128, 48], DT, kind="ExternalInput")
    wcd = nc.dram_tensor("wc", [128, NK * CH], DTW, kind="ExternalInput")
    out = nc.dram_tensor("out", [CH], DT, kind="ExternalOutput")

    rg = [list(range(NCORES))]
    Act = mybir.ActivationFunctionType

    with tile.TileContext(nc) as tc:
        with (
            tc.tile_pool(name="weights", bufs=1) as wp,
            tc.tile_pool(name="small", bufs=1) as sp,
            tc.tile_pool(name="act", bufs=1) as ap,
            tc.tile_pool(name="psum", bufs=1, space=bass.MemorySpace.PSUM) as pp,
            tc.tile_pool(name="dram", bufs=1, space=bass.MemorySpace.DRAM) as dp,
        ):
            # ---- weight streams (HWDGE, 384 KB chunks) ----
            wt = []
            for c in range(4):
                t_ = wp.tile([128, CHUNK], DT8, tag=f"w0c{c}", name=f"w0c{c}")
                nc.sync.dma_start(t_[:], w0d[:, c * CHUNK:(c + 1) * CHUNK])
                wt.append(t_)
            vt = []
            for c in range(4):
                t_ = wp.tile([128, CHUNK], DT8, tag=f"w1c{c}", name=f"w1c{c}")
                nc.sync.dma_start(t_[:], w1d[:, c * CHUNK:(c + 1) * CHUNK])
                vt.append(t_)

            # ---- small loads (SWDGE) + on-chip constants ----
            x0sb = sp.tile([128, NK], DTW, tag="x0")
            nc.gpsimd.dma_start(x0sb[:], x0d[:])
            p0c = sp.tile([1, 787], DT, tag="p0c")
            nc.gpsimd.dma_start(p0c[:], p0cd[:])
            biasg = sp.tile([128, 48], DT, tag="biasg")
            nc.gpsimd.dma_start(biasg[:], biasgd[:])
            wcsb = sp.tile([128, NK * CH], DTW, tag="wc")
            nc.gpsimd.dma_start(wcsb[:], wcd[:])
            ones32 = sp.tile([1, 1], DT, tag="ones32")
            nc.gpsimd.memset(ones32[:], 1.0)
            ones16 = sp.tile([128, 1], DTW, tag="ones16")
            nc.gpsimd.memset(ones16[:], 1.0)
            scratch = sp.tile([128, 512], DTW, tag="scratch")
            nc.gpsimd.memset(scratch[:], 0.0)

            # ---- PE warm-up: ~4.3us of dummy matmuls flips HAM to 2.4 GHz
            # while the first weight chunk is still in flight ----
            psW = pp.tile([1, 512], DT, tag="psW")
            for _ in range(10):
                nc.tensor.matmul(psW[:], ones16[:, 0:1], scratch[:],
                                 start=True, stop=True)

            # ---- layer 0: flipped GEMV, 4x column-tiled ----
            # group g -> psum partition 32g, cols [i(64)|o(64)|g(64)]
            psL0 = pp.tile([128, 192], DT, tag="psL0")
            for t in range(NK):
                ch = wt[t // 4]
                base = (t % 4) * 768
                for g in range(4):
                    nc.tensor.matmul(
                        psL0[32 * g:32 * g + 1, :], x0sb[:, t:t + 1],
                        ch[:, base + g * 192: base + (g + 1) * 192],
                        start=(t == 0), stop=False, tile_position=(0, 32 * g))
            for g in range(4):
                nc.tensor.matmul(
                    psL0[32 * g:32 * g + 1, :], ones32[:],
                    p0c[0:1, g * 192:(g + 1) * 192],
                    start=False, stop=True, tile_position=(0, 32 * g))

            # ---- layer-0 LSTM activations (garbage lanes compute for free) --
            sig_io = ap.tile([128, 128], DT, tag="sig_io")
            nc.scalar.activation(sig_io[:], psL0[:, 0:128], Act.Sigmoid,
                                 scale=1.0 / S)
            tanh_g = ap.tile([128, 64], DT, tag="tanh_g")
            nc.scalar.activation(tanh_g[:], psL0[:, 128:192], Act.Tanh,
                                 scale=1.0 / S)
            cst = ap.tile([128, 64], DT, tag="cst")
            nc.vector.tensor_mul(cst[:], sig_io[:, 0:64], tanh_g[:])
            tanh_c = ap.tile([128, 64], DT, tag="tanh_c")
            nc.scalar.activation(tanh_c[:], cst[:], Act.Tanh)
            h0 = ap.tile([128, 64], DTW, tag="h0")
            nc.vector.tensor_mul(h0[:], tanh_c[:], sig_io[:, 64:128])

            # ---- transpose h0 [4x64] -> x1 [128,2] via K=1 matmuls ----
            psT = pp.tile([128, 2], DT, tag="psT")
            for g in range(4):
                pb, cc = 64 * (g % 2), g // 2
                nc.tensor.matmul(
                    psT[pb:pb + 64, cc:cc + 1], h0[32 * g:32 * g + 1, :],
                    ones16[32 * g:32 * g + 1, :], start=True, stop=True,
                    tile_position=(32 * g, pb))
            x1 = ap.tile([128, 2], DTW, tag="x1")
            nc.vector.tensor_copy(x1[:], psT[:])

            # ---- layer 1: partial gates, 4x column-tiled ----
            # psM[m] partition 32g col j = partial gate f = g*1536 + m*512 + j
            psM = [pp.tile([128, 512], DT, tag=f"psM{m}", name=f"psM{m}")
                   for m in range(3)]
            for c in range(2):
                for g in range(4):
                    for m in range(3):
                        col = c * G1 + g * 1536 + m * 512
                        nc.tensor.matmul(
                            psM[m][32 * g:32 * g + 1, :], x1[:, c:c + 1],
                            vt[col // CHUNK][:, col % CHUNK: col % CHUNK + 512],
                            start=(c == 0), stop=(c == 1),
                            tile_position=(0, 32 * g))

            partials = sp.tile([128, 1536], DTW, tag="partials")
            nc.vector.tensor_copy(partials[:, 0:512], psM[0][:])
            nc.vector.tensor_copy(partials[:, 512:1024], psM[1][:])
            nc.scalar.activation(partials[:, 1024:1536], psM[2][:], Act.Copy)

            # ---- single bf16 AllGather of the partials ----
            cc_in = dp.tile([G1], DTW, tag="cc_in")
            nc.scalar.dma_start(
                cc_in.rearrange("(g j) -> g j", g=4), partials[0:128:32, :])
            cc_out = dp.tile([NCORES * G1], DTW, tag="cc_out")
            nc.gpsimd.collective_compute(
                "AllGather", mybir.AluOpType.bypass,
                ins=[cc_in.opt()], outs=[cc_out.opt()], replica_groups=rg,
            )

            # ---- readback [128, 8, 48] (two queues) + tree reduce + bias ----
            R = sp.tile([128, NCORES, 48], DTW, tag="R")
            rb = cc_out.rearrange("(r p j) -> p r j", r=NCORES, p=128)
            nc.sync.dma_start(R[:, 0:4, :], rb[:, 0:4, :])
            nc.scalar.dma_start(R[:, 4:8, :], rb[:, 4:8, :])
            S1 = ap.tile([128, 4, 48], DT, tag="S1")
            nc.vector.tensor_add(S1[:], R[:, 0:4, :], R[:, 4:8, :])
            S2 = ap.tile([128, 2, 48], DT, tag="S2")
            nc.vector.tensor_add(S2[:], S1[:, 0:2, :], S1[:, 2:4, :])
            S3 = ap.tile([128, 48], DT, tag="S3")
            nc.vector.tensor_add(S3[:], S2[:, 0, :], S2[:, 1, :])
            G = ap.tile([128, 48], DT, tag="G")
            nc.vector.tensor_add(G[:], S3[:], biasg[:])

            # ---- layer-1 LSTM activations across 128 partitions ----
            sig_io1 = ap.tile([128, 32], DT, tag="sig_io1")
            nc.scalar.activation(sig_io1[:], G[:, 0:32], Act.Sigmoid,
                                 scale=1.0 / S)
            tanh_g1 = ap.tile([128, 16], DT, tag="tanh_g1")
            nc.scalar.activation(tanh_g1[:], G[:, 32:48], Act.Tanh,
                                 scale=1.0 / S)
            cst1 = ap.tile([128, 16], DT, tag="cst1")
            nc.vector.tensor_mul(cst1[:], sig_io1[:, 0:16], tanh_g1[:])
            tanh_c1 = ap.tile([128, 16], DT, tag="tanh_c1")
            nc.scalar.activation(tanh_c1[:], cst1[:], Act.Tanh)
            h1 = ap.tile([128, 16], DTW, tag="h1")
            nc.vector.tensor_mul(h1[:], tanh_c1[:], sig_io1[:, 16:32])

            # ---- choice head: logits [1,19] = h1 . Wc + bc ----
            psH = pp.tile([1, CH], DT, tag="psH")
            for t in range(NK):
                nc.tensor.matmul(psH[:], h1[:, t:t + 1],
                                 wcsb[:, t * CH:(t + 1) * CH],
                                 start=(t == 0), stop=False)
            nc.tensor.matmul(psH[:], ones32[:], p0c[0:1, 768:787],
                             start=False, stop=True)
            logit = ap.tile([1, CH], DT, tag="logit")
            nc.vector.tensor_copy(logit[:], psH[:])
            nc.scalar.dma_start(out.rearrange("(a n) -> a n", a=1), logit[:])

    nc.compile()
    return nc


_NC_CACHE = None


def _get_nc():
    global _NC_CACHE
    if _NC_CACHE is None:
        _NC_CACHE = _build_nc()
    return _NC_CACHE


# --------------------------------------------------------------------------
# entry point
# --------------------------------------------------------------------------

def kernel(**inputs) -> np.ndarray:
    task = int(np.asarray(inputs["task"]).reshape(-1)[0]) if not isinstance(
        inputs["task"], int) else int(inputs["task"])
    maps = _host_prep(inputs)
    nc = _get_nc()
    for attempt in range(3):
        res = run_bass_kernel_spmd(nc, maps, list(range(NCORES)))
        outs = [np.asarray(res.results[i]["out"], np.float32).reshape(CH)
                for i in range(NCORES)]
        # post-AllGather every core holds identical logits; disagreement means
        # the device was in a bad state -- retry
        if all(np.array_equal(outs[0], o) for o in outs[1:]):
            break
    logits = outs[0]
    mask = np.arange(CH) < (1 + task)
    return np.where(mask, logits, np.float32(-1e9)).astype(np.float32)


if __name__ == "__main__":
    import reference  # only for standalone debugging; not used by the grader

    inputs = reference.setup_inputs()
    expected = np.asarray(reference.reference(**inputs))
    actual = kernel(**inputs)
    print("expected:", expected)
    print("actual:  ", actual)
    denom = np.abs(expected).max()
    print("max abs err:", np.abs(actual - expected).max(),
          "rel:", np.abs(actual - expected).max() / denom)


# revision 7
# speedup vs baseline: 1.0833x; 1.0833x over previous
"""Trainium2 Bass kernel for nn_Controller (batch-1 two-layer LSTM-cell chain
+ choice head), distributed over 8 NeuronCores.

Math notes: both LSTMCells run with zero initial state, so h @ W_hh.T == 0 and
the f-gate multiplies c=0.  Only the i/g/o thirds of each W_ih matter:
    gates = x @ W_ih.T + (b_ih + b_hh)
    h     = sigmoid(o) * tanh(sigmoid(i) * tanh(g))

Structure (one collective total):
  - Weights are stored fp8-e4m3 pre-scaled by S=128 on the host (keeps the
    0.02-sigma values out of the subnormal range); the matching 1/S descale is
    folded into the activation-function `scale` operand for free.
  - Layer 0 row-sharded: core k computes the 768 i/o/g gate rows of its h0
    chunk as a flipped GEMV (x column stationary -> 1-column weight loads)
    using 4x PE column-group tiling: groups at array cols {0,32,64,96} stream
    four weight tiles concurrently, landing gates on psum partitions
    {0,32,64,96}.  Bias rides the accumulation group via K=1 matmuls vs ones.
  - h0 [4x64] is transposed to contraction layout [128,2] with four K=1
    matmuls against a ones column.
  - Layer 1 contraction-sharded: core k multiplies its own h0 chunk into
    W_ih_1[:, chunk] producing partial pre-activations for ALL 6144 gates
    (again 4x column-tiled).  No inter-layer collective.
  - One bf16 AllGather of the partials; each core tree-reduces the 8 copies
    on DVE (fp32), adds bias, applies the LSTM activations across 128
    partitions, and computes the 19-logit head locally.
  - ~10 warm-up matmuls on scratch data run during the initial weight-DMA
    window so the PE HAM clock-gate is at 2.4 GHz when real work arrives.
"""

import os
import sys

import numpy as np
import ml_dtypes

for _p in ("/opt/trn_rl_repo", os.path.expanduser("~/.axon_site/_ro/trn_rl_repo")):
    if os.path.isdir(_p) and _p not in sys.path:
        sys.path.insert(0, _p)

import concourse.bass as bass
import concourse.bacc as bacc
import concourse.mybir as mybir
import concourse.tile as tile
from concourse.bass_utils import run_bass_kernel_spmd

H = 2048
NCORES = 8
C = H // NCORES          # 256: per-core h chunk
NK = H // 128            # 16 k-tiles
CH = 19                  # choice logits
G1 = 3 * H               # 6144 layer-1 gates (i,o,g)
S = 128.0                # fp8 weight pre-scale
DT = mybir.dt.float32
DTW = mybir.dt.bfloat16
DT8 = mybir.dt.float8e4
BF = ml_dtypes.bfloat16
F8 = ml_dtypes.float8_e4m3

W0_COLS = NK * 768               # fp8 cols, 4 chunks of 3072
W1_COLS = 2 * G1                 # c-major: col = c*6144 + g*1536 + m*512 + j
CHUNK = 3072
# gate order everywhere is (i, o, g) so the two sigmoids fuse into one op
AROW = np.array([0, 3 * H, 2 * H])


# --------------------------------------------------------------------------
# host-side layout prep
# --------------------------------------------------------------------------

def _l0_rows(k):
    """W_ih_0 rows for core k in psum-column order: 4 col-groups x
    [i(64)|o(64)|g(64)], group g covering local h0 elems 64g..64g+63."""
    out = []
    for g in range(4):
        e = k * C + 64 * g + np.arange(64)
        out.append(np.concatenate([e, 3 * H + e, 2 * H + e]))
    return np.concatenate(out)


def _l1_rows():
    """W_ih_1 rows in AllGather-buffer order f = g*1536 + m*512 + j, where
    post-readback G[p, a*16+t] = gate a of h1 elem t*128+p, a in (i,o,g)."""
    f = np.arange(G1)
    p, rem = f // 48, f % 48
    a, t = rem // 16, rem % 16
    return AROW[a] + t * 128 + p


def _host_prep(inputs):
    idx = int(np.asarray(inputs["input_idx"]).reshape(-1)[0])
    emb = np.asarray(inputs["embedding"], np.float32)
    x0 = emb[idx]
    x0T = np.ascontiguousarray(x0.reshape(NK, 128).T.astype(BF))  # [128,16]

    W0 = np.asarray(inputs["w_ih_0"], np.float32)
    W1 = np.asarray(inputs["w_ih_1"], np.float32)
    B0 = np.asarray(inputs["b_ih_0"], np.float32) + np.asarray(inputs["b_hh_0"], np.float32)
    B1 = np.asarray(inputs["b_ih_1"], np.float32) + np.asarray(inputs["b_hh_1"], np.float32)
    WC = np.asarray(inputs["w_choice"], np.float32)
    BC = np.asarray(inputs["b_choice"], np.float32)

    frows = _l1_rows()
    W1g = (W1[frows] * S).astype(F8)                     # [6144, 2048] fp8
    biasg = np.ascontiguousarray((B1[frows] * S).reshape(128, 48))

    wch = np.ascontiguousarray(
        np.transpose(WC.reshape(CH, NK, 128), (2, 1, 0)).reshape(128, NK * CH)
        .astype(BF))

    maps = []
    for k in range(NCORES):
        r0 = _l0_rows(k)
        blk = (W0[r0] * S).T.reshape(NK, 128, 768)       # [t, p, n] f32
        w0h = np.ascontiguousarray(
            np.transpose(blk, (1, 0, 2)).reshape(128, W0_COLS).astype(F8))

        b0h = np.ascontiguousarray(
            np.concatenate([B0[r0] * S, BC]).reshape(1, 787))

        # w1h[q, c*6144 + f] = S*W1[frows[f], k*256 + c*128 + q]
        sel = W1g[:, k * C:(k + 1) * C]                  # [6144, 256] fp8
        arr = sel.reshape(G1, 2, 128)                    # [f, c, q]
        w1h = np.ascontiguousarray(
            np.transpose(arr, (2, 1, 0)).reshape(128, W1_COLS))

        maps.append(dict(w0=w0h, w1=w1h, x0=x0T, p0c=b0h, biasg=biasg, wc=wch))
    return maps


# --------------------------------------------------------------------------
# device program (identical on all 8 cores; per-core data differs)
# --------------------------------------------------------------------------

def _build_nc():
    nc = bacc.Bacc("TRN2", target_bir_lowering=False, debug=False,
                   num_devices=NCORES)

    w0d = nc.dram_tensor("w0", [128, W0_COLS], DT8, kind="ExternalInput")
    w1d = nc.dram_tensor("w1", [128, W1_COLS], DT8, kind="ExternalInput")
    x0d = nc.dram_tensor("x0", [128, NK], DTW, kind="ExternalInput")
    p0cd = nc.dram_tensor("p0c", [1, 787], DT, kind="ExternalInput")
    biasgd = nc.dram_tensor("biasg", [128, 48], DT, kind="ExternalInput")
    wcd = nc.dram_tensor("wc", [128, NK * CH], DTW, kind="ExternalInput")
    out = nc.dram_tensor("out", [CH], DT, kind="ExternalOutput")

    rg = [list(range(NCORES))]
    Act = mybir.ActivationFunctionType

    with tile.TileContext(nc) as tc:
        with (
            tc.tile_pool(name="weights", bufs=1) as wp,
            tc.tile_pool(name="small", bufs=1) as sp,
            tc.tile_pool(name="act", bufs=1) as ap,
            tc.tile_pool(name="psum", bufs=1, space=bass.MemorySpace.PSUM) as pp,
            tc.tile_pool(name="dram", bufs=1, space=bass.MemorySpace.DRAM) as dp,
        ):
            # ---- weight streams (HWDGE, 384 KB chunks) ----
            wt = []
            for c in range(4):
                t_ = wp.tile([128, CHUNK], DT8, tag=f"w0c{c}", name=f"w0c{c}")
                nc.sync.dma_start(t_[:], w0d[:, c * CHUNK:(c + 1) * CHUNK])
                wt.append(t_)
            vt = []
            for c in range(4):
                t_ = wp.tile([128, CHUNK], DT8, tag=f"w1c{c}", name=f"w1c{c}")
                nc.sync.dma_start(t_[:], w1d[:, c * CHUNK:(c + 1) * CHUNK])
                vt.append(t_)

            # ---- small loads (SWDGE) + on-chip constants ----
            x0sb = sp.tile([128, NK], DTW, tag="x0")
            nc.gpsimd.dma_start(x0sb[:], x0d[:])
            p0c = sp.tile([1, 787], DT, tag="p0c")
            nc.gpsimd.dma_start(p0c[:], p0cd[:])
            biasg = sp.tile([128, 48], DT, tag="biasg")
            nc.gpsimd.dma_start(biasg[:], biasgd[:])
            wcsb = sp.tile([128, NK * CH], DTW, tag="wc")
            nc.gpsimd.dma_start(wcsb[:], wcd[:])
            ones32 = sp.tile([1, 1], DT, tag="ones32")
            nc.gpsimd.memset(ones32[:], 1.0)
            ones16 = sp.tile([128, 1], DTW, tag="ones16")
            nc.gpsimd.memset(ones16[:], 1.0)
            # ---- layer 0: flipped GEMV, 4x column-tiled ----
            # group g -> psum partition 32g, cols [i(64)|o(64)|g(64)]
            psL0 = pp.tile([128, 192], DT, tag="psL0")
            for t in range(NK):
                ch = wt[t // 4]
                base = (t % 4) * 768
                for g in range(4):
                    nc.tensor.matmul(
                        psL0[32 * g:32 * g + 1, :], x0sb[:, t:t + 1],
                        ch[:, base + g * 192: base + (g + 1) * 192],
                        start=(t == 0), stop=False, tile_position=(0, 32 * g))
            for g in range(4):
                nc.tensor.matmul(
                    psL0[32 * g:32 * g + 1, :], ones32[:],
                    p0c[0:1, g * 192:(g + 1) * 192],
                    start=False, stop=True, tile_position=(0, 32 * g))

            # ---- layer-0 LSTM activations (garbage lanes compute for free) --
            sig_io = ap.tile([128, 128], DT, tag="sig_io")
            nc.scalar.activation(sig_io[:], psL0[:, 0:128], Act.Sigmoid,
                                 scale=1.0 / S)
            tanh_g = ap.tile([128, 64], DT, tag="tanh_g")
            nc.scalar.activation(tanh_g[:], psL0[:, 128:192], Act.Tanh,
                                 scale=1.0 / S)
            cst = ap.tile([128, 64], DT, tag="cst")
            nc.vector.tensor_mul(cst[:], sig_io[:, 0:64], tanh_g[:])
            tanh_c = ap.tile([128, 64], DT, tag="tanh_c")
            nc.scalar.activation(tanh_c[:], cst[:], Act.Tanh)
            h0 = ap.tile([128, 64], DTW, tag="h0")
            nc.vector.tensor_mul(h0[:], tanh_c[:], sig_io[:, 64:128])

            # ---- transpose h0 [4x64] -> x1 [128,2] via K=1 matmuls ----
            psT = pp.tile([128, 2], DT, tag="psT")
            for g in range(4):
                pb, cc = 64 * (g % 2), g // 2
                nc.tensor.matmul(
                    psT[pb:pb + 64, cc:cc + 1], h0[32 * g:32 * g + 1, :],
                    ones16[32 * g:32 * g + 1, :], start=True, stop=True,
                    tile_position=(32 * g, pb))
            x1 = ap.tile([128, 2], DTW, tag="x1")
            nc.vector.tensor_copy(x1[:], psT[:])

            # ---- layer 1: partial gates, 4x column-tiled ----
            # psM[m] partition 32g col j = partial gate f = g*1536 + m*512 + j
            psM = [pp.tile([128, 512], DT, tag=f"psM{m}", name=f"psM{m}")
                   for m in range(3)]
            for c in range(2):
                for g in range(4):
                    for m in range(3):
                        col = c * G1 + g * 1536 + m * 512
                        nc.tensor.matmul(
                            psM[m][32 * g:32 * g + 1, :], x1[:, c:c + 1],
                            vt[col // CHUNK][:, col % CHUNK: col % CHUNK + 512],
                            start=(c == 0), stop=(c == 1),
                            tile_position=(0, 32 * g))

            partials = sp.tile([128, 1536], DTW, tag="partials")
            nc.vector.tensor_copy(partials[:, 0:512], psM[0][:])
            nc.vector.tensor_copy(partials[:, 512:1024], psM[1][:])
            nc.scalar.activation(partials[:, 1024:1536], psM[2][:], Act.Copy)

            # ---- single bf16 AllGather of the partials ----
            cc_in = dp.tile([G1], DTW, tag="cc_in")
            nc.scalar.dma_start(
                cc_in.rearrange("(g j) -> g j", g=4), partials[0:128:32, :])
            cc_out = dp.tile([NCORES * G1], DTW, tag="cc_out",
                             addr_space="Shared")
            nc.gpsimd.collective_compute(
                "AllGather", mybir.AluOpType.bypass,
                ins=[cc_in.opt()], outs=[cc_out.opt()], replica_groups=rg,
            )

            # ---- readback [128, 8, 48] (three queues) + tree reduce + bias --
            R = sp.tile([128, NCORES, 48], DTW, tag="R")
            rb = cc_out.rearrange("(r p j) -> p r j", r=NCORES, p=128)
            nc.sync.dma_start(R[:, 0:3, :], rb[:, 0:3, :])
            nc.scalar.dma_start(R[:, 3:6, :], rb[:, 3:6, :])
            nc.gpsimd.dma_start(R[:, 6:8, :], rb[:, 6:8, :])
            S1 = ap.tile([128, 4, 48], DT, tag="S1")
            nc.vector.tensor_add(S1[:], R[:, 0:4, :], R[:, 4:8, :])
            S2 = ap.tile([128, 2, 48], DT, tag="S2")
            nc.vector.tensor_add(S2[:], S1[:, 0:2, :], S1[:, 2:4, :])
            S3 = ap.tile([128, 48], DT, tag="S3")
            nc.vector.tensor_add(S3[:], S2[:, 0, :], S2[:, 1, :])
            G = ap.tile([128, 48], DT, tag="G")
            nc.vector.tensor_add(G[:], S3[:], biasg[:])

            # ---- layer-1 LSTM activations across 128 partitions ----
            sig_io1 = ap.tile([128, 32], DT, tag="sig_io1")
            nc.scalar.activation(sig_io1[:], G[:, 0:32], Act.Sigmoid,
                                 scale=1.0 / S)
            tanh_g1 = ap.tile([128, 16], DT, tag="tanh_g1")
            nc.scalar.activation(tanh_g1[:], G[:, 32:48], Act.Tanh,
                                 scale=1.0 / S)
            cst1 = ap.tile([128, 16], DT, tag="cst1")
            nc.vector.tensor_mul(cst1[:], sig_io1[:, 0:16], tanh_g1[:])
            tanh_c1 = ap.tile([128, 16], DT, tag="tanh_c1")
            nc.scalar.activation(tanh_c1[:], cst1[:], Act.Tanh)
            h1 = ap.tile([128, 16], DTW, tag="h1")
            nc.vector.tensor_mul(h1[:], tanh_c1[:], sig_io1[:, 16:32])

            # ---- choice head: logits [1,19] = h1 . Wc + bc ----
            psH = pp.tile([1, CH], DT, tag="psH")
            for t in range(NK):
                nc.tensor.matmul(psH[:], h1[:, t:t + 1],
                                 wcsb[:, t * CH:(t + 1) * CH],
                                 start=(t == 0), stop=False)
            nc.tensor.matmul(psH[:], ones32[:], p0c[0:1, 768:787],
                             start=False, stop=True)
            logit = ap.tile([1, CH], DT, tag="logit")
            nc.vector.tensor_copy(logit[:], psH[:])
            nc.scalar.dma_start(out.rearrange("(a n) -> a n", a=1), logit[:])

    nc.compile()
    return nc


_NC_CACHE = None


def _get_nc():
    global _NC_CACHE
    if _NC_CACHE is None:
        _NC_CACHE = _build_nc()
    return _NC_CACHE


# --------------------------------------------------------------------------
# entry point
# --------------------------------------------------------------------------

def kernel(**inputs) -> np.ndarray:
    task = int(np.asarray(inputs["task"]).reshape(-1)[0]) if not isinstance(
        inputs["task"], int) else int(inputs["task"])
    maps = _host_prep(inputs)
    nc = _get_nc()
    for attempt in range(3):
        res = run_bass_kernel_spmd(nc, maps, list(range(NCORES)))
        outs = [np.asarray(res.results[i]["out"], np.float32).reshape(CH)
                for i in range(NCORES)]
        # post-AllGather every core holds identical logits; disagreement means
        # the device was in a bad state -- retry
        if all(np.array_equal(outs[0], o) for o in outs[1:]):
            break
    logits = outs[0]
    mask = np.arange(CH) < (1 + task)
    return np.where(mask, logits, np.float32(-1e9)).astype(np.float32)


if __name__ == "__main__":
    import reference  # only for standalone debugging; not used by the grader

    inputs = reference.setup_inputs()
    expected = np.asarray(reference.reference(**inputs))
    actual = kernel(**inputs)
    print("expected:", expected)
    print("actual:  ", actual)
    denom = np.abs(expected).max()
    print("max abs err:", np.abs(actual - expected).max(),
          "rel:", np.abs(actual - expected).max() / denom)
